# revision 26
# baseline (speedup 1.0000x reference)
import sys

sys.path.insert(0, "/opt/trn_rl_repo")

from contextlib import ExitStack

import numpy as np

import concourse.bacc as bacc
import concourse.mybir as mybir
from concourse import tile
from concourse.bass_utils import run_bass_kernel_spmd

F32 = mybir.dt.float32
F32R = mybir.dt.float32r
AL = mybir.AluOpType
AF = mybir.ActivationFunctionType

C = 256
H = W = 64
NC = 8  # cores / batch shards


# ---------------------------------------------------------------- host prep
def tf32_round(x):
    u = np.ascontiguousarray(np.asarray(x, np.float32)).view(np.uint32)
    u = (u + np.uint32(1 << 12) + ((u >> 13) & np.uint32(1))).astype(np.uint32) \
        & np.uint32(0xFFFFE000)
    return u.view(np.float32)


def host_prep(inp):
    """Rearrange all weights into [partition, free] layouts matching SBUF tiles."""
    d = {}
    f = np.float32

    # conditioning nets (dsc1, dsc2)
    for i, pre in ((0, "dsc1"), (1, "dsc2")):
        w1 = np.asarray(inp[f"{pre}_w1"], f)  # [64, 256]
        b1 = np.asarray(inp[f"{pre}_b1"], f)  # [64]
        w2 = np.asarray(inp[f"{pre}_w2"], f)  # [2304, 64]
        b2 = np.asarray(inp[f"{pre}_b2"], f)  # [2304]
        # lhsT for gm matmul: [k_local, chunk, m]; fold the 1/(H*W) mean here
        d[f"w1T{i}"] = np.ascontiguousarray(
            (w1.T / (H * W)).reshape(2, 128, 64).transpose(1, 0, 2)
        ).reshape(128, 128)
        d[f"b1_{i}"] = b1.reshape(64, 1).copy()
        d[f"b1s{i}"] = (0.7978845608028654 * b1).reshape(64, 1).copy()
        # lhsT for wts matmul: [j, chunk, k, c_local]; gelu's 0.5 folded in.
        # row 64 = b2 folded in via a constant-1.0 element appended to hv.
        w2m = np.ascontiguousarray(
            (0.5 * w2).reshape(2, 128, 9, 64).transpose(3, 0, 2, 1)
        ).reshape(64, 2304)
        b2row = np.ascontiguousarray(
            b2.reshape(2, 128, 9).transpose(0, 2, 1)
        ).reshape(1, 2304)  # [(c, k, m)]
        import ml_dtypes as _mldw
        d[f"w2r{i}"] = np.concatenate([w2m, b2row], axis=0).astype(
            _mldw.bfloat16)  # [65, 2304]

    # channel_align 1x1: [k_local, kc, mc, m]  (f32r single pass)
    aw = np.asarray(inp["align_w"], f)[:, :, 0, 0]  # [256, 512]
    import ml_dtypes as _mlda
    d["alignw"] = np.ascontiguousarray(
        aw.reshape(2, 128, 4, 128).transpose(3, 2, 0, 1)
    ).reshape(128, 1024).astype(_mlda.bfloat16)
    d["alignb"] = np.ascontiguousarray(
        np.asarray(inp["align_b"], f).reshape(2, 128).T
    )  # [128, 2]

    # fused upsampler: (1x1 up2) o (pixel shuffle) o (3x3 up1) == per output
    # phase p a 3x3 conv 256->128: Wp[m,c,dy,dx] = sum_g u2[m,g] u1[4g+p,c,dy,dx]
    u1 = np.asarray(inp["up_w1"], np.float64)  # [1024, 256, 3, 3]
    u2 = np.asarray(inp["up_w2"], np.float64)[:, :, 0, 0]  # [128, 256]
    b1u = np.asarray(inp["up_b1"], np.float64)  # [1024]
    b2u = np.asarray(inp["up_b2"], np.float64)  # [128]
    u1r = u1.reshape(256, 4, 256, 3, 3)  # [g, p, c, dy, dx]
    wp = np.einsum("mg,gpcyx->pyxcm", u2, u1r)  # [p, dy, dx, c, m]
    # layout [k_local=128, (p, tap, kc, m)]
    wp = wp.reshape(4, 3, 3, 2, 128, 128).transpose(4, 0, 1, 2, 3, 5)
    wp = np.ascontiguousarray(wp).reshape(128, 9216) * 64.0  # /64 at psum drain
    import ml_dtypes
    e4 = lambda a: np.asarray(a, np.float32).astype(ml_dtypes.float8_e4m3fn)
    wh = e4(wp)
    d["fupw8h"] = wh
    d["fupw8l"] = e4(wp - wh.astype(np.float64))
    bp = b2u[None, :] + np.einsum("mg,gp->pm", u2, b1u.reshape(256, 4))  # [4, 128]

    # ---- polyphase re_enhance ----
    def split(v):  # v = r + dy - 1
        rp = v % 2
        return rp, (v - rp) // 2

    r1w = np.asarray(inp["re_w1"], f)  # [32, 128, 3, 3]
    re2b_vec = np.asarray(inp["re_b2"], np.float64)  # [128]
    # up2p carries +re2b so the re2 drain can spend its scalar slot on the
    # 1/64 fp8-weight descale; up8h subtracts it back out for re1's rhs.
    bp = bp + re2b_vec[None, :]
    d["fupb"] = np.ascontiguousarray(bp.T.astype(f))  # [128, 4]
    keymap = {}
    for p in range(4):
        r, s = p // 2, p % 2
        for dy in range(3):
            for dx in range(3):
                rp, qy = split(r + dy - 1)
                sp, qx = split(s + dx - 1)
                keymap.setdefault((2 * rp + sp, qy, qx), []).append((p, dy, dx))
    keys = sorted(keymap.keys(), key=lambda k: (k[1] != 0 or k[2] != 0, k))
    re1_keys = keys  # list of (p_in, qy, qx); all-(0,0) shifts first
    re1w = np.zeros((128, 16, 128), np.float64)
    for ki, key in enumerate(keys):
        for (p, dy, dx) in keymap[key]:
            re1w[:, ki, p * 32: (p + 1) * 32] = r1w[:, :, dy, dx].T
    re1w = re1w * 64.0  # /64 at the psum drain
    import ml_dtypes as _mld1
    _e41 = lambda a: np.asarray(a, np.float32).astype(_mld1.float8_e4m3fn)
    w1h = _e41(re1w)
    w1l = _e41(re1w - w1h.astype(np.float64))
    # same-qy key pairs; all three passes use cross-key pairs so no rhs
    # needs a zero-stride pair dim (hw-safe). zero-weight partner if odd.
    groups = {}
    for ki, key in enumerate(keys):
        groups.setdefault(key[1], []).append((key[0], key[2], ki))
    re1_p3 = []  # (qy, (pinA,qxA,kiA), (pinB,qxB,kiB) | None)
    w3h, w3l = [], []
    z = np.zeros((128, 128), w1h.dtype)
    for qy in sorted(groups, key=lambda q: q != 0):
        g = sorted(groups[qy])
        while g:
            a = g.pop(0)
            b = g.pop(0) if g else None
            re1_p3.append((qy, a, b))
            wbh = w1h[:, b[2], :] if b is not None else z
            wbl = w1l[:, b[2], :] if b is not None else z
            w3h.append(np.concatenate([w1h[:, a[2], None, :], wbh[:, None, :]], axis=1))
            w3l.append(np.concatenate([w1l[:, a[2], None, :], wbl[:, None, :]], axis=1))
    # single-pass fp8 DoubleRow weights: [128, pair_i * (2*128)], x64 scale
    # (descale at the psum drain); rhs is the fp8 image of up.
    d["re1w8"] = np.ascontiguousarray(
        np.concatenate([w.reshape(128, 256) for w in w3h], axis=1))
    d["re1b"] = np.tile(np.asarray(inp["re_b1"], f), 4).reshape(128, 1)
    d["re1_p3"] = re1_p3

    r2w = np.asarray(inp["re_w2"], f)  # [128, 32, 3, 3]
    # pairs: per out-phase, two qy groups x (qxA, qxB); qy=0 group first
    re2_pairs = []  # per phase: [(qy, qxA, qxB), (qy2, qxA, qxB)]
    re2w = np.zeros((128, 4, 2, 2, 128), np.float64)  # [k, p, pair, k2, m]
    for p in range(4):
        r, s = p // 2, p % 2
        qys = sorted({split(r + dy - 1)[1] for dy in range(3)}, key=lambda q: q != 0)
        qxs = sorted({split(s + dx - 1)[1] for dx in range(3)})
        prs = []
        for pj, qy in enumerate(qys):
            prs.append((qy, qxs[0], qxs[1]))
            for k2, qx in enumerate(qxs):
                for pp in range(4):
                    rp, sp = pp // 2, pp % 2
                    dy = 2 * qy + rp - r + 1
                    dx = 2 * qx + sp - s + 1
                    if 0 <= dy < 3 and 0 <= dx < 3:
                        re2w[pp * 32: (pp + 1) * 32, p, pj, k2, :] = r2w[:, :, dy, dx].T
        re2_pairs.append(prs)
    import ml_dtypes as _mld
    _e4 = lambda a: np.asarray(a, np.float32).astype(_mld.float8_e4m3fn)
    d["re2w8"] = _e4((re2w * 64.0).reshape(128, 2048))  # /64 at psum drain
    d["re2b"] = np.asarray(inp["re_b2"], f).reshape(128, 1).copy()
    d["re2_pairs"] = re2_pairs

    import ml_dtypes as _mldi
    d["ident"] = np.eye(128).astype(_mldi.float8_e4m3fn)
    re2_pairs = d.pop("re2_pairs")
    re1_p3 = d.pop("re1_p3")
    return d, (re1_keys, re1_p3), re2_pairs


RE1_KEYS = None
RE2_Q = None


def _mapping():
    global RE1_KEYS, RE2_Q
    if RE1_KEYS is None:
        zeros = {k: np.zeros(v) for k, v in [
            ("dsc1_w1", (64, 256)), ("dsc1_b1", (64,)), ("dsc1_w2", (2304, 64)),
            ("dsc1_b2", (2304,)), ("dsc2_w1", (64, 256)), ("dsc2_b1", (64,)),
            ("dsc2_w2", (2304, 64)), ("dsc2_b2", (2304,)),
            ("align_w", (256, 512, 1, 1)), ("align_b", (256,)),
            ("up_w1", (1024, 256, 3, 3)), ("up_b1", (1024,)),
            ("up_w2", (128, 256, 1, 1)), ("up_b2", (128,)),
            ("re_w1", (32, 128, 3, 3)), ("re_b1", (32,)),
            ("re_w2", (128, 32, 3, 3)), ("re_b2", (128,)),
        ]}
        _, RE1_KEYS, RE2_Q = host_prep(zeros)
    return RE1_KEYS, RE2_Q


# ---------------------------------------------------------------- bass build
def pimg(ap):
    """View of a column-padded [128, 64*66] image (zero cols at x=0 and x=65)."""
    return ap.rearrange("p (y x) -> p y x", x=66)


def mm_views(src_ap, psum_ap, sy, sx, n):
    """(psum_out, rhs) for 8-row chunk n of a shifted conv tap on a col-padded
    source. Row range restricted by sy; columns handled by the zero pad.
    psum out is a contiguous 2D region."""
    y0 = max(0, -sy)
    y1 = 64 + min(0, -sy)
    r0 = max(8 * n, y0)
    r1 = min(8 * n + 8, y1)
    if r1 <= r0:
        return None, None
    v = pimg(src_ap)
    rhs = v[:, r0 + sy: r1 + sy, 1 + sx: 65 + sx]
    out = psum_ap[:, (r0 - 8 * n) * 64: (r1 - 8 * n) * 64]
    return out, rhs


def build():
    (re1_keys, re1_p3), re2_pairs = _mapping()
    nc = bacc.Bacc(trn_type="TRN2", target_bir_lowering=False, debug=False)

    BF16 = mybir.dt.bfloat16
    x_d = [nc.dram_tensor(n, [256, 4096], BF16, kind="ExternalInput")
           for n in ("x1", "x2")]
    wd = {}
    for name, shape, dt in [
        ("w1T0", [128, 128], F32), ("w1T1", [128, 128], F32),
        ("b1_0", [64, 1], F32), ("b1_1", [64, 1], F32),
        ("b1s0", [64, 1], F32), ("b1s1", [64, 1], F32),
        ("w2r0", [65, 2304], mybir.dt.bfloat16), ("w2r1", [65, 2304], mybir.dt.bfloat16),
        ("alignw", [128, 1024], mybir.dt.bfloat16), ("alignb", [128, 2], F32),
        ("fupw8h", [128, 9216], mybir.dt.float8e4), ("fupb", [128, 4], F32),
        ("fupw8l", [128, 9216], mybir.dt.float8e4),
        ("re1w8", [128, 2048], mybir.dt.float8e4), ("re1b", [128, 1], F32),
        ("re2w8", [128, 2048], mybir.dt.float8e4), ("re2b", [128, 1], F32),
        ("ident", [128, 128], mybir.dt.float8e4),
    ]:
        wd[name] = nc.dram_tensor(name, shape, dt, kind="ExternalInput")
    out_d = nc.dram_tensor("out", [128, 16384], mybir.dt.bfloat16,
                           kind="ExternalOutput")

    with tile.TileContext(nc) as tc, ExitStack() as ctx:
        wpool = ctx.enter_context(tc.tile_pool(name="w", bufs=1))
        big = ctx.enter_context(tc.tile_pool(name="big", bufs=5))
        flatp = ctx.enter_context(tc.tile_pool(name="flat", bufs=4))
        bnd = ctx.enter_context(tc.tile_pool(name="bnd", bufs=4))
        dgp = ctx.enter_context(tc.tile_pool(name="dg", bufs=2))
        f8p = ctx.enter_context(tc.tile_pool(name="f8", bufs=1))
        tiny = ctx.enter_context(tc.tile_pool(name="tiny", bufs=4))
        ps = ctx.enter_context(tc.tile_pool(name="ps", bufs=8, space="PSUM"))

        # persistent weights, spread across the three HWDGE queues (SP /
        # Activation / DVE) in rough order of first use so nothing blocks
        # the x loads or the first conditioning pass.
        wt = {}
        wq = {
            "w1T0": nc.sync, "b1_0": nc.sync, "b1s0": nc.sync, "w2r0": nc.sync,
            "ident": nc.scalar, "alignb": nc.gpsimd, "fupb": nc.gpsimd,
            "re1b": nc.gpsimd, "re2b": nc.gpsimd, "alignw": nc.gpsimd,
            "w1T1": nc.sync, "b1_1": nc.sync, "b1s1": nc.sync, "w2r1": nc.sync,
            "fupw8h": nc.gpsimd, "fupw8l": nc.gpsimd,
            "re1w8": nc.gpsimd, "re2w8": nc.gpsimd,
        }
        for name in wq:
            if name == "w2r1":
                continue  # allocated lazily into w2r0's slot (same tag)
            tag = "w2r0" if name == "w2r0" else name
            t = wpool.tile(list(wd[name].shape), wd[name].dtype, tag=tag)
            wt[name] = t

        def padtile(tag="big"):
            """Fresh [128, 64*66] F32R tile with zeroed pad columns."""
            t = big.tile([128, 4224], F32R, tag=tag)
            v = pimg(t[:].bitcast(F32))
            nc.vector.memset(v[:, :, 0:1], 0.0)
            nc.vector.memset(v[:, :, 65:66], 0.0)
            return t

        # x in: contiguous quarter DMAs into small flat tiles (16KB-contig
        # descriptors dodge the <512B strided-DMA 2x penalty), then ScalarE
        # repacks each quarter into the padded layout — with accum_out
        # producing the per-channel sums as a side effect.
        xq = [[nc.sync, nc.gpsimd], [nc.sync, nc.gpsimd]]
        xin = [[padtile(), padtile()], [padtile(), padtile()]]
        xgs = [[None, None], [None, None]]

        def load_x(i, c, q):
            xf = flatp.tile([128, 1024], mybir.dt.bfloat16, tag="flat")
            xq[i][c].dma_start(xf[:],
                               x_d[i].ap()[c * 128: (c + 1) * 128,
                                           q * 1024: (q + 1) * 1024])
            dst = pimg(xin[i][c][:])[:, q * 16: (q + 1) * 16, 1:65]
            src = xf[:].rearrange("p (y x) -> p y x", x=64)
            if (i, c) == (0, 0):
                # ScalarE repack; accum_out yields the channel sums for free
                if xgs[i][c] is None:
                    gq = tiny.tile([128, 4], F32, tag="gs4")
                    xgs[i][c] = gq
                nc.scalar.activation(dst, src, AF.Copy,
                                     accum_out=xgs[i][c][:, q: q + 1])
            elif (i, c) == (0, 1):
                # x1c1 on DVE, in parallel with ScalarE's x1c0 chain; the
                # cheap flat-quarter reduce gates gms, the repack follows
                if xgs[i][c] is None:
                    gq1 = tiny.tile([128, 4], F32, tag="gs4")
                    xgs[i][c] = gq1
                nc.vector.tensor_reduce(xgs[i][c][:, q: q + 1], xf[:],
                                        axis=mybir.AxisListType.X, op=AL.add)
                nc.vector.tensor_scalar_add(dst, src, 0.0)
            else:
                # x2 repacks on Pool, which is idle during dw1(0,*); DVE is
                # carrying depthwise taps and x2 gms reduces then
                nc.gpsimd.tensor_scalar_add(dst, src, 0.0)

        gms1 = [[None, None], [None, None]]

        def gms_of(i, c):
            g = tiny.tile([128, 1], F32, tag="gms")
            if i == 0:
                nc.vector.tensor_reduce(g[:], xgs[i][c][:],
                                        axis=mybir.AxisListType.X, op=AL.add)
            else:
                nc.vector.tensor_reduce(g[:], xin[i][c][:].bitcast(F32),
                                        axis=mybir.AxisListType.X, op=AL.add)
            gms1[i][c] = g

        # All x quarters go through the SP queue: its SEQ serializes them in
        # this order, which also orders them on the (serial) DMA engines —
        # x1 lands first, x2 right behind, nothing jumps the line.
        # high_priority pins the x1 chain ahead of weight-DMA configs in
        # every scheduler tie so the extra flat buffers can't be hijacked.
        with tc.high_priority():
            for q in range(4):
                load_x(0, 0, q)
                load_x(0, 1, q)
        # conditioning weights for dsc1 right behind x1
        for name in ("w1T0", "b1_0", "b1s0", "w2r0", "ident"):
            wq[name].dma_start(wt[name][:], wd[name].ap())
        gms_of(0, 0)
        gms_of(0, 1)

        # ---------------- conditioning ----------------
        def cond_pg(d, gms):
            """gms: two [128,1] sums -> hv65 [65,1]: gelu'd hidden (x0.5
            folded) rows 0-63 plus a constant 1.0 row for the b2 fold."""
            pgt = ps.tile([128, 512], F32, tag="ps")
            pg = pgt[0:64, 0:1]
            for c in range(2):
                nc.tensor.matmul(pg, wt[f"w1T{d}"][:, c * 64: (c + 1) * 64],
                                 gms[c][:], start=(c == 0), stop=(c == 1))
            # u is O(1e-2) so the cubic term is negligible:
            # gelu(u) ~= 0.5 u (1 + tanh(0.79788 u)); the 0.5 lives in w2r.
            # Tanh shares the Exp act table -> no LoadActFuncSet swaps.
            th = tiny.tile([64, 1], F32, tag="th")
            nc.scalar.activation(th[:], pg, AF.Tanh, scale=0.7978845608028654,
                                 bias=wt[f"b1s{d}"][:])
            thp = tiny.tile([64, 1], F32, tag="thp")
            nc.vector.tensor_scalar_add(thp[:], th[:], 1.0)
            hv = tiny.tile([65, 1], mybir.dt.bfloat16, tag="hv")
            nc.vector.memset(hv[64:65, :], 1.0)
            nc.vector.scalar_tensor_tensor(hv[0:64, :], pg, wt[f"b1_{d}"][:],
                                           thp[:], AL.add, AL.mult)
            return hv

        def cond_pw(d, hv):
            """-> per chunk (ex [128,9] unnormalized softmax weights,
            c4 [128,1] = ex4+ssum center tap, rec [128,1] = 1/ssum).
            Normalization by rec happens in the psum-drain copy's scale."""
            res = []
            for c in range(2):
                pw = ps.tile([128, 512], F32, tag="ps")
                for k in range(9):
                    nc.tensor.matmul(pw[:, k: k + 1],
                                     wt[f"w2r{d}"][:, (c * 9 + k) * 128: (c * 9 + k + 1) * 128],
                                     hv[:], start=True, stop=True)
                # logits are O(1e-3) here (0.02-scale weights, gelu'd tiny
                # hidden): exp() cannot overflow, so no max subtraction.
                ex = tiny.tile([128, 9], F32, tag="ex")
                ssum = tiny.tile([128, 1], F32, tag="ssum")
                with tc.high_priority():
                    nc.scalar.activation(ex[:], pw[:, 0:9], AF.Exp, accum_out=ssum[:])
                rec = tiny.tile([128, 1], F32, tag="rec")
                c4 = tiny.tile([128, 1], F32, tag="c4")
                with tc.high_priority():
                    nc.vector.reciprocal(rec[:], ssum[:])
                    nc.vector.tensor_tensor(c4[:], ex[:, 4:5], ssum[:], AL.add)
                res.append((ex, c4, rec))
            return res

        def build_diags(ex, c4, rec):
            """center diag (its own tile so PE's first tap only waits on it)
            + 8 side diags split ScalarE/DVE; entries unnormalized (rec is
            applied at the psum drain)."""
            dgc = dgp.tile([128, 128], F32R, tag="dgc")
            dg = dgp.tile([128, 1024], F32R, tag="dg")
            with tc.high_priority():
                nc.scalar.activation(dgc[:], wt["ident"][:], AF.Copy, scale=c4[:])
                for j, k in enumerate((0, 1, 2, 3, 5, 6, 7, 8)):
                    dst = dg[:, j * 128: (j + 1) * 128]
                    if j % 2 == 0:
                        nc.scalar.activation(dst, wt["ident"][:], AF.Copy,
                                             scale=ex[:, k: k + 1])
                    else:
                        nc.vector.tensor_scalar(dst, wt["ident"][:],
                                                ex[:, k: k + 1], None, AL.mult)
            return dgc, dg

        SIDE = [0, 1, 2, 3, 5, 6, 7, 8]

        def depthwise_pe(src, dgs, dst_write, skip=(), mid_hook=None):
            """9-tap depthwise conv + residual on PE via diagonal matmuls.
            dst_write(n, psum): copy band n of the accumulated result out.
            Taps in `skip` are left for another engine to add afterwards.
            mid_hook() is emitted after band 3 so its PE ops run mid-chunk."""
            dgc, dg = dgs
            side = [k for k in SIDE if k not in skip]
            for n in range(8):
                if n == 4 and mid_hook is not None:
                    mid_hook()
                p = ps.tile([128, 512], F32, tag="ps")
                nc.tensor.matmul(p[:], dgc[:],
                                 pimg(src[:])[:, n * 8: (n + 1) * 8, 1:65],
                                 start=True, stop=False, skip_group_check=True)
                for ji, k in enumerate(side):
                    sy, sx = k // 3 - 1, k % 3 - 1
                    o, rhs = mm_views(src[:], p[:], sy, sx, n)
                    nc.tensor.matmul(o, dg[:, SIDE.index(k) * 128: (SIDE.index(k) + 1) * 128],
                                     rhs, start=False, stop=(ji == len(side) - 1),
                                     skip_group_check=True)
                dst_write(n, p)

        # ---------------- dsc stage ----------------
        # Emission order matters: PE executes its queue in order, so the tiny
        # conditioning matmuls for the NEXT (input, layer) are emitted between
        # depthwise chunks — their DVE/ScalarE chains then overlap the current
        # depthwise instead of stalling PE.
        mid = [[None, None], [None, None]]
        gms2 = [[None, None], [None, None]]
        y = [[None, None], [None, None]]

        def dw1(i, c, cw, mid_hook=None):
            dgs = build_diags(*cw[c])
            ex, _, rec = cw[c]
            m = padtile()
            gs = tiny.tile([128, 8], F32, tag="gs")
            src_v = pimg(xin[i][c][:])

            def wr(n, p):
                # tap 7 (sy=+1) added by DVE straight into the psum between
                # the PE group and the ScalarE drain (unnormalized, like
                # PE's); for x2 (i=1) DVE also takes tap 1 — its repack work
                # is done by then. GPSIMD legally cannot touch PSUM.
                # high_priority: beat the x2 repacks for the DVE.
                pv = p[:].rearrange("p (y x) -> p y x", x=64)
                r0, r1 = n * 8, min((n + 1) * 8, 63)
                if r1 > r0:
                    with tc.high_priority():
                        nc.vector.scalar_tensor_tensor(
                            pv[:, 0: r1 - r0, :],
                            src_v[:, r0 + 1: r1 + 1, 1:65].bitcast(F32),
                            ex[:, 7: 8], pv[:, 0: r1 - r0, :], AL.mult, AL.add)
                if i == 1:
                    q0, q1 = max(n * 8, 1), (n + 1) * 8
                    with tc.high_priority():
                        nc.vector.scalar_tensor_tensor(
                            pv[:, q0 - n * 8: q1 - n * 8, :],
                            src_v[:, q0 - 1: q1 - 1, 1:65].bitcast(F32),
                            ex[:, 1: 2], pv[:, q0 - n * 8: q1 - n * 8, :],
                            AL.mult, AL.add)
                nc.scalar.activation(pimg(m[:])[:, n * 8: (n + 1) * 8, 1: 65],
                                     p[:].rearrange("p (y x) -> p y x", y=8),
                                     AF.Relu, scale=rec[:], accum_out=gs[:, n: n + 1])

            depthwise_pe(xin[i][c], dgs, wr, skip=(1, 7) if i == 1 else (7,),
                         mid_hook=mid_hook)
            g2 = tiny.tile([128, 1], F32, tag="gms")
            nc.vector.tensor_reduce(g2[:], gs[:], axis=mybir.AxisListType.X, op=AL.add)
            mid[i][c] = m
            gms2[i][c] = g2

        def dw2(i, c, cw):
            dgs = build_diags(*cw[c])
            ex, _, rec = cw[c]
            yt = big.tile([128, 4096], mybir.dt.bfloat16, tag="big")

            def wr(n, p):
                nc.scalar.activation(yt[:, n * 512: (n + 1) * 512], p[:], AF.Copy,
                                     scale=rec[:])

            # taps 1/7 (sy=+-1) on DVE and tap 3 (sx=-1) on Pool, right
            # behind each ScalarE psum drain
            wns = {}
            for k in (1, 3, 7):
                wn = tiny.tile([128, 1], F32, tag="wn")
                nc.vector.tensor_scalar_mul(wn[:], ex[:, k: k + 1], rec[:])
                wns[k] = wn
            yv = yt[:].rearrange("p (y x) -> p y x", x=64)
            mv = pimg(mid[i][c][:])

            def wr2(n, p):
                wr(n, p)
                for k in (1, 7):
                    sy = k // 3 - 1
                    r0 = max(n * 8, -sy)
                    r1 = min((n + 1) * 8, 64 - max(0, sy))
                    nc.vector.scalar_tensor_tensor(
                        yv[:, r0: r1, :],
                        mv[:, r0 + sy: r1 + sy, 1:65].bitcast(F32),
                        wns[k][:], yv[:, r0: r1, :], AL.mult, AL.add)
                nc.gpsimd.scalar_tensor_tensor(
                    yv[:, n * 8: (n + 1) * 8, :],
                    mv[:, n * 8: (n + 1) * 8, 0:64].bitcast(F32),
                    wns[3][:], yv[:, n * 8: (n + 1) * 8, :], AL.mult, AL.add)

            depthwise_pe(mid[i][c], dgs, wr2, skip=(1, 3, 7))
            y[i][c] = yt

        hv = cond_pg(0, gms1[0])
        # x2 loads + sums, queued behind x1's
        for q in range(4):
            load_x(1, 0, q)
            load_x(1, 1, q)
        gms_of(1, 0)
        gms_of(1, 1)
        # Remaining weights held back (scheduler-time wait) so their
        # transfers cannot claim the serial DMA engines before the x loads.
        with tc.tile_wait_until(0.03):
            for name in ("w1T1", "b1_1", "b1s1", "alignb", "fupb",
                         "re1b", "re2b", "alignw", "re1w8", "re2w8",
                         "fupw8h", "fupw8l"):
                nc.scalar.dma_start(wt[name][:], wd[name].ap())
        cw = cond_pw(0, hv)
        dw1(0, 0, cw)
        hvb = cond_pg(0, gms1[1])
        dw1(0, 1, cw)
        cwb = cond_pw(0, hvb)
        # dsc2's w2r reuses dsc1's buffer now that its readers are emitted
        w2r1t = wpool.tile([65, 2304], mybir.dt.bfloat16, tag="w2r0")
        wt["w2r1"] = w2r1t
        nc.scalar.dma_start(w2r1t[:], wd["w2r1"].ap())
        dw1(1, 0, cwb)
        hv = cond_pg(1, gms2[0])       # gms2[0] ready during dw1(1,0)
        cw = cond_pw(1, hv)
        dw1(1, 1, cwb)
        dw2(0, 0, cw)
        hvb = cond_pg(1, gms2[1])
        dw2(0, 1, cw)
        cwb = cond_pw(1, hvb)
        dw2(1, 0, cwb)
        dw2(1, 1, cwb)

        # ---------------- align 1x1 (2C -> C) -> fp8 hi/lo fus versions ----------------
        # fus is stored as three e4m3 images, both kc chunks in ONE tile
        # (kc-major) so a DoubleRow rhs AP can address the pair:
        #   fus8h = e4m3(A), fus8s = e4m3(A)/16, fus8l = A - e4m3(A)
        FP8 = mybir.dt.float8e4

        def fp8img(tag):
            t = f8p.tile([128, 8448], FP8, tag=tag)
            v = t[:].rearrange("p (kc y x) -> p kc y x", kc=2, x=66)
            nc.vector.memset(v[:, :, :, 0:1], 0.0)
            nc.vector.memset(v[:, :, :, 65:66], 0.0)
            return t

        def fp8img1(tag):
            t = f8p.tile([128, 4224], FP8, tag=tag)
            v = t[:].rearrange("p (y x) -> p y x", x=66)
            nc.vector.memset(v[:, :, 0:1], 0.0)
            nc.vector.memset(v[:, :, 65:66], 0.0)
            return t

        fus8h = fp8img("f8h")
        fus8l = fp8img("f8l")
        for mc in range(2):
            for n in range(8):
                p = ps.tile([128, 512], F32, tag="ps")
                for kc in range(4):
                    rhs = y[kc // 2][kc % 2][:, n * 512: (n + 1) * 512]
                    nc.tensor.matmul(
                        p[:], wt["alignw"][:, (kc * 2 + mc) * 128: (kc * 2 + mc + 1) * 128],
                        rhs, start=(kc == 0), stop=(kc == 3))
                pv = p[:].rearrange("p (y x) -> p y x", y=8)
                roff = mc * 4224 + n * 8 * 66
                hv_ = fus8h[:, roff: roff + 528].rearrange("p (y x) -> p y x", x=66)[:, :, 1:65]
                lv_ = fus8l[:, roff: roff + 528].rearrange("p (y x) -> p y x", x=66)[:, :, 1:65]
                nc.scalar.activation(hv_, pv, AF.Identity, bias=wt["alignb"][:, mc: mc + 1])
                nc.vector.scalar_tensor_tensor(lv_, pv, wt["alignb"][:, mc: mc + 1],
                                               hv_, AL.add, AL.subtract)

        # ---------------- fused upsampler: per-phase 3x3 conv 256->128 ----------------
        # fp8 e4m3 DoubleRow (both kc in one matmul, 0.5 cyc/row), 3 passes:
        #   Wh@Ah + (Wl*16)@(Ah/16) + Wh@(A-Ah); weights carry x64, /64 at drain
        taps = [(1, 1)] + [(dy, dx) for dy in range(3) for dx in range(3) if (dy, dx) != (1, 1)]

        def dr_views(src_ap, psum_ap, sy, sx, n):
            y0 = max(0, -sy)
            y1 = 64 + min(0, -sy)
            r0 = max(8 * n, y0)
            r1 = min(8 * n + 8, y1)
            v = src_ap.rearrange("p (kc y x) -> p kc y x", kc=2, x=66)
            rhs = v[:, :, r0 + sy: r1 + sy, 1 + sx: 65 + sx]
            out = psum_ap[:, (r0 - 8 * n) * 64: (r1 - 8 * n) * 64]
            return out, rhs

        def pair_ap(base3d, delta):
            u = base3d.unsqueeze(1).copy()
            u.ap[1] = [delta, 2]
            return u

        up2p = []
        up8 = []
        for p4 in range(4):
            up2p.append(padtile())
            up8.append(fp8img1(f"u8{p4}"))

        def fup_band(p4, n):
            p = ps.tile([128, 512], F32, tag="ps")
            first = True
            for (dy, dx) in taps:
                sy, sx = dy - 1, dx - 1
                toff = (p4 * 9 + dy * 3 + dx) * 256
                wh = wt["fupw8h"][:, toff: toff + 256].rearrange(
                    "p (k m) -> p k m", k=2)
                wl = wt["fupw8l"][:, toff: toff + 256].rearrange(
                    "p (k m) -> p k m", k=2)
                for wtile, src in ((wh, fus8h), (wl, fus8h), (wh, fus8l)):
                    o, rhs = dr_views(src[:], p[:], sy, sx, n)
                    nc.tensor.matmul(o, wtile, rhs, start=first,
                                     stop=((dy, dx) == taps[-1] and src is fus8l),
                                     perf_mode=mybir.MatmulPerfMode.DoubleRow,
                                     skip_group_check=True)
                    first = False
            pv = p[:].rearrange("p (y x) -> p y x", y=8)
            fb = pimg(up2p[p4][:])[:, n * 8: (n + 1) * 8, 1:65]
            nc.scalar.activation(fb, pv, AF.Identity, scale=0.015625,
                                 bias=wt["fupb"][:, p4: p4 + 1])
            # fp8 copy of up (re2b backed out) for re1's DoubleRow rhs;
            # runs on DVE, which is otherwise idle during the fup window
            fbf = pimg(up2p[p4][:].bitcast(F32))[:, n * 8: (n + 1) * 8, 1:65]
            u8b = pimg(up8[p4][:])[:, n * 8: (n + 1) * 8, 1:65]
            nc.vector.tensor_scalar(u8b, fbf, wt["re2b"][:], None,
                                    AL.subtract)

        # ---------------- re1 (polyphase 3x3, M-packed, fp8 DR 1-pass) ----------------
        s64 = tiny.tile([128, 1], F32, tag="s64")
        nc.vector.memset(s64[:], 0.015625)

        re1t = fp8img1("re18")

        def re1_n(n):
            p = ps.tile([128, 512], F32, tag="ps")
            for pi, (qy, pa, pb) in enumerate(re1_p3):
                pinA, qxA, _ = pa
                pinB, qxB, _ = pb
                assert pinB == pinA and qxB - qxA == 1
                o, rhs = mm_views(up8[pinA][:], p[:], qy, qxA, n)
                u = rhs.unsqueeze(1).copy()
                u.ap[1] = [1, 2]
                nc.tensor.matmul(o, wt["re1w8"][:, pi * 256: (pi + 1) * 256]
                                 .rearrange("p (k m) -> p k m", k=2),
                                 u, start=(pi == 0), stop=(pi == len(re1_p3) - 1),
                                 perf_mode=mybir.MatmulPerfMode.DoubleRow,
                                 skip_group_check=True)
            pv = p[:].rearrange("p (y x) -> p y x", y=8)
            fb = pimg(re1t[:])[:, n * 8: (n + 1) * 8, 1:65]
            nc.scalar.activation(fb, pv, AF.Relu, scale=0.015625,
                                 bias=wt["re1b"][:])

        # ---------------- re2 (polyphase 3x3, fp8 DR 1-pass) + residual + out ----------------
        def re2_n(n, split=False):
            pss = []
            for p4 in range(4):
                p = ps.tile([128, 512], F32, tag="ps")
                for pj, (qy, qxA, qxB) in enumerate(re2_pairs[p4]):
                    assert qxB - qxA == 1
                    o, rhs = mm_views(re1t[:], p[:], qy, qxA, n)
                    u = rhs.unsqueeze(1).copy()
                    u.ap[1] = [1, 2]
                    nc.tensor.matmul(
                        o, wt["re2w8"][:, (p4 * 2 + pj) * 256:
                                       (p4 * 2 + pj + 1) * 256]
                        .rearrange("p (k m) -> p k m", k=2),
                        u, start=(pj == 0), stop=(pj == 1),
                        perf_mode=mybir.MatmulPerfMode.DoubleRow,
                        skip_group_check=True)
                pss.append(p)
            band = bnd.tile([128, 2048], mybir.dt.bfloat16, tag="bnd")
            bv = band[:].rearrange("p (y r x s) -> p y r x s", y=8, r=2, s=2)
            if not split:
                for p4 in range(4):
                    r, s = p4 // 2, p4 % 2
                    up_v = pimg(up2p[p4][:].bitcast(F32))[:, n * 8: (n + 1) * 8, 1:65]
                    out_v = bv[:, :, r, :, s]
                    psv = pss[p4][:].rearrange("p (y x) -> p y x", y=8)
                    if p4 < 3:
                        # up2p carries +re2b: psum/64 + up2p is the full output
                        nc.vector.scalar_tensor_tensor(
                            out_v, psv, s64[:], up_v, AL.mult, AL.add)
                    else:
                        # ScalarE drains psum/64; Pool adds the residual (+re2b)
                        nc.scalar.activation(out_v, psv, AF.Identity,
                                             scale=0.015625)
                        nc.gpsimd.tensor_tensor(out_v, out_v, up_v, AL.add)
                nc.sync.dma_start(
                    out_d.ap()[:, n * 2048: (n + 1) * 2048], band[:])
                return
            # tail bands: half-band drains spread across DVE/ACT/Pool and two
            # half DMAs, so the final store only waits on the last half
            for h in range(2):
                for p4 in range(4):
                    r, s = p4 // 2, p4 % 2
                    up_v = pimg(up2p[p4][:].bitcast(F32))[
                        :, n * 8 + h * 4: n * 8 + (h + 1) * 4, 1:65]
                    out_v = bv[:, h * 4: (h + 1) * 4, r, :, s]
                    psv = pss[p4][:].rearrange("p (y x) -> p y x", y=8)[
                        :, h * 4: (h + 1) * 4, :]
                    if p4 < 2:
                        nc.vector.scalar_tensor_tensor(
                            out_v, psv, s64[:], up_v, AL.mult, AL.add)
                    else:
                        nc.scalar.activation(out_v, psv, AF.Identity,
                                             scale=0.015625)
                        if p4 == 2:
                            nc.gpsimd.tensor_tensor(out_v, out_v, up_v, AL.add)
                        else:
                            nc.vector.tensor_tensor(out_v, out_v, up_v, AL.add)
                nc.sync.dma_start(
                    out_d.ap()[:, n * 2048 + h * 1024: n * 2048 + (h + 1) * 1024],
                    band[:, h * 1024: (h + 1) * 1024])

        # band-major driver: all four fup phases of band n, then re1(n-1)
        # and re2(n-2) — each re stage trails by one band because it reads
        # its input through row 8n+8. Spreading re1/re2 into the fup window
        # lets their drain chains ride on otherwise-idle DVE/ACT/Pool time
        # instead of forming a drain-bound tail phase.
        for n in range(8):
            for p4 in range(4):
                fup_band(p4, n)
            if n >= 1:
                re1_n(n - 1)
            if n >= 2:
                re2_n(n - 2)
        re1_n(7)
        re2_n(6, split=True)
        re2_n(7, split=True)

    nc.compile()
    return nc


_NC = None


def _get_nc():
    global _NC
    if _NC is None:
        _NC = build()
    return _NC


def make_in_maps(inputs):
    import ml_dtypes
    w, _, _ = host_prep(inputs)
    x1 = np.ascontiguousarray(np.asarray(inputs["x1"], np.float32).reshape(NC, 256, 4096)
                              .astype(ml_dtypes.bfloat16))
    x2 = np.ascontiguousarray(np.asarray(inputs["x2"], np.float32).reshape(NC, 256, 4096)
                              .astype(ml_dtypes.bfloat16))
    in_maps = []
    for i in range(NC):
        m = {"x1": x1[i], "x2": x2[i]}
        m.update(w)
        in_maps.append(m)
    return in_maps


def kernel(**inputs):
    nc = _get_nc()
    in_maps = make_in_maps(inputs)
    res = run_bass_kernel_spmd(nc, in_maps, core_ids=list(range(NC)))
    out = np.stack([res.results[i]["out"].reshape(128, 128, 128) for i in range(NC)])
    return out.astype(np.float32)



# revision 34
# speedup vs baseline: 1.0176x; 1.0176x over previous
import sys

sys.path.insert(0, "/opt/trn_rl_repo")

from contextlib import ExitStack

import numpy as np

import concourse.bacc as bacc
import concourse.mybir as mybir
from concourse import tile
from concourse.bass_utils import run_bass_kernel_spmd

F32 = mybir.dt.float32
F32R = mybir.dt.float32r
AL = mybir.AluOpType
AF = mybir.ActivationFunctionType

C = 256
H = W = 64
NC = 8  # cores / batch shards


# ---------------------------------------------------------------- host prep
def tf32_round(x):
    u = np.ascontiguousarray(np.asarray(x, np.float32)).view(np.uint32)
    u = (u + np.uint32(1 << 12) + ((u >> 13) & np.uint32(1))).astype(np.uint32) \
        & np.uint32(0xFFFFE000)
    return u.view(np.float32)


def host_prep(inp):
    """Rearrange all weights into [partition, free] layouts matching SBUF tiles."""
    d = {}
    f = np.float32

    # conditioning nets (dsc1, dsc2)
    for i, pre in ((0, "dsc1"), (1, "dsc2")):
        w1 = np.asarray(inp[f"{pre}_w1"], f)  # [64, 256]
        b1 = np.asarray(inp[f"{pre}_b1"], f)  # [64]
        w2 = np.asarray(inp[f"{pre}_w2"], f)  # [2304, 64]
        b2 = np.asarray(inp[f"{pre}_b2"], f)  # [2304]
        # lhsT for gm matmul: [k_local, chunk, m]; fold the 1/(H*W) mean here
        d[f"w1T{i}"] = np.ascontiguousarray(
            (w1.T / (H * W)).reshape(2, 128, 64).transpose(1, 0, 2)
        ).reshape(128, 128)
        d[f"b1_{i}"] = b1.reshape(64, 1).copy()
        d[f"b1s{i}"] = (0.7978845608028654 * b1).reshape(64, 1).copy()
        # lhsT for wts matmul: [j, chunk, k, c_local]; gelu's 0.5 folded in.
        # row 64 = b2 folded in via a constant-1.0 element appended to hv.
        w2m = np.ascontiguousarray(
            (0.5 * w2).reshape(2, 128, 9, 64).transpose(3, 0, 2, 1)
        ).reshape(64, 2304)
        b2row = np.ascontiguousarray(
            b2.reshape(2, 128, 9).transpose(0, 2, 1)
        ).reshape(1, 2304)  # [(c, k, m)]
        import ml_dtypes as _mldw
        d[f"w2r{i}"] = np.concatenate([w2m, b2row], axis=0).astype(
            _mldw.bfloat16)  # [65, 2304]

    # channel_align 1x1: [k_local, kc, mc, m]  (f32r single pass)
    aw = np.asarray(inp["align_w"], f)[:, :, 0, 0]  # [256, 512]
    import ml_dtypes as _mlda
    d["alignw"] = np.ascontiguousarray(
        aw.reshape(2, 128, 4, 128).transpose(3, 2, 0, 1)
    ).reshape(128, 1024).astype(_mlda.bfloat16)
    d["alignb"] = np.ascontiguousarray(
        np.asarray(inp["align_b"], f).reshape(2, 128).T
    )  # [128, 2]

    # fused upsampler: (1x1 up2) o (pixel shuffle) o (3x3 up1) == per output
    # phase p a 3x3 conv 256->128: Wp[m,c,dy,dx] = sum_g u2[m,g] u1[4g+p,c,dy,dx]
    u1 = np.asarray(inp["up_w1"], np.float64)  # [1024, 256, 3, 3]
    u2 = np.asarray(inp["up_w2"], np.float64)[:, :, 0, 0]  # [128, 256]
    b1u = np.asarray(inp["up_b1"], np.float64)  # [1024]
    b2u = np.asarray(inp["up_b2"], np.float64)  # [128]
    u1r = u1.reshape(256, 4, 256, 3, 3)  # [g, p, c, dy, dx]
    wp = np.einsum("mg,gpcyx->pyxcm", u2, u1r)  # [p, dy, dx, c, m]
    # layout [k_local=128, (p, tap, kc, m)]
    wp = wp.reshape(4, 3, 3, 2, 128, 128).transpose(4, 0, 1, 2, 3, 5)
    wp = np.ascontiguousarray(wp).reshape(128, 9216) * 64.0  # /64 at psum drain
    import ml_dtypes
    e4 = lambda a: np.asarray(a, np.float32).astype(ml_dtypes.float8_e4m3fn)
    wh = e4(wp)
    d["fupw8h"] = wh
    d["fupw8l"] = e4(wp - wh.astype(np.float64))
    bp = b2u[None, :] + np.einsum("mg,gp->pm", u2, b1u.reshape(256, 4))  # [4, 128]

    # ---- polyphase re_enhance ----
    def split(v):  # v = r + dy - 1
        rp = v % 2
        return rp, (v - rp) // 2

    r1w = np.asarray(inp["re_w1"], f)  # [32, 128, 3, 3]
    re2b_vec = np.asarray(inp["re_b2"], np.float64)  # [128]
    # up2p carries +re2b so the re2 drain can spend its scalar slot on the
    # 1/64 fp8-weight descale; up8h subtracts it back out for re1's rhs.
    bp = bp + re2b_vec[None, :]
    d["fupb"] = np.ascontiguousarray(bp.T.astype(f))  # [128, 4]
    keymap = {}
    for p in range(4):
        r, s = p // 2, p % 2
        for dy in range(3):
            for dx in range(3):
                rp, qy = split(r + dy - 1)
                sp, qx = split(s + dx - 1)
                keymap.setdefault((2 * rp + sp, qy, qx), []).append((p, dy, dx))
    keys = sorted(keymap.keys(), key=lambda k: (k[1] != 0 or k[2] != 0, k))
    re1_keys = keys  # list of (p_in, qy, qx); all-(0,0) shifts first
    re1w = np.zeros((128, 16, 128), np.float64)
    for ki, key in enumerate(keys):
        for (p, dy, dx) in keymap[key]:
            re1w[:, ki, p * 32: (p + 1) * 32] = r1w[:, :, dy, dx].T
    re1w = re1w * 64.0  # /64 at the psum drain
    import ml_dtypes as _mld1
    _e41 = lambda a: np.asarray(a, np.float32).astype(_mld1.float8_e4m3fn)
    w1h = _e41(re1w)
    w1l = _e41(re1w - w1h.astype(np.float64))
    # same-qy key pairs; all three passes use cross-key pairs so no rhs
    # needs a zero-stride pair dim (hw-safe). zero-weight partner if odd.
    groups = {}
    for ki, key in enumerate(keys):
        groups.setdefault(key[1], []).append((key[0], key[2], ki))
    re1_p3 = []  # (qy, (pinA,qxA,kiA), (pinB,qxB,kiB) | None)
    w3h, w3l = [], []
    z = np.zeros((128, 128), w1h.dtype)
    for qy in sorted(groups, key=lambda q: q != 0):
        g = sorted(groups[qy])
        while g:
            a = g.pop(0)
            b = g.pop(0) if g else None
            re1_p3.append((qy, a, b))
            wbh = w1h[:, b[2], :] if b is not None else z
            wbl = w1l[:, b[2], :] if b is not None else z
            w3h.append(np.concatenate([w1h[:, a[2], None, :], wbh[:, None, :]], axis=1))
            w3l.append(np.concatenate([w1l[:, a[2], None, :], wbl[:, None, :]], axis=1))
    # single-pass fp8 DoubleRow weights: [128, pair_i * (2*128)], x64 scale
    # (descale at the psum drain); rhs is the fp8 image of up.
    d["re1w8"] = np.ascontiguousarray(
        np.concatenate([w.reshape(128, 256) for w in w3h], axis=1))
    d["re1b"] = np.tile(np.asarray(inp["re_b1"], f), 4).reshape(128, 1)
    d["re1_p3"] = re1_p3

    r2w = np.asarray(inp["re_w2"], f)  # [128, 32, 3, 3]
    # pairs: per out-phase, two qy groups x (qxA, qxB); qy=0 group first
    re2_pairs = []  # per phase: [(qy, qxA, qxB), (qy2, qxA, qxB)]
    re2w = np.zeros((128, 4, 2, 2, 128), np.float64)  # [k, p, pair, k2, m]
    for p in range(4):
        r, s = p // 2, p % 2
        qys = sorted({split(r + dy - 1)[1] for dy in range(3)}, key=lambda q: q != 0)
        qxs = sorted({split(s + dx - 1)[1] for dx in range(3)})
        prs = []
        for pj, qy in enumerate(qys):
            prs.append((qy, qxs[0], qxs[1]))
            for k2, qx in enumerate(qxs):
                for pp in range(4):
                    rp, sp = pp // 2, pp % 2
                    dy = 2 * qy + rp - r + 1
                    dx = 2 * qx + sp - s + 1
                    if 0 <= dy < 3 and 0 <= dx < 3:
                        re2w[pp * 32: (pp + 1) * 32, p, pj, k2, :] = r2w[:, :, dy, dx].T
        re2_pairs.append(prs)
    import ml_dtypes as _mld
    _e4 = lambda a: np.asarray(a, np.float32).astype(_mld.float8_e4m3fn)
    d["re2w8"] = _e4((re2w * 64.0).reshape(128, 2048))  # /64 at psum drain
    d["re2b"] = np.asarray(inp["re_b2"], f).reshape(128, 1).copy()
    d["re2_pairs"] = re2_pairs

    import ml_dtypes as _mldi
    d["ident"] = np.eye(128).astype(_mldi.float8_e4m3fn)
    re2_pairs = d.pop("re2_pairs")
    re1_p3 = d.pop("re1_p3")
    return d, (re1_keys, re1_p3), re2_pairs


RE1_KEYS = None
RE2_Q = None


def _mapping():
    global RE1_KEYS, RE2_Q
    if RE1_KEYS is None:
        zeros = {k: np.zeros(v) for k, v in [
            ("dsc1_w1", (64, 256)), ("dsc1_b1", (64,)), ("dsc1_w2", (2304, 64)),
            ("dsc1_b2", (2304,)), ("dsc2_w1", (64, 256)), ("dsc2_b1", (64,)),
            ("dsc2_w2", (2304, 64)), ("dsc2_b2", (2304,)),
            ("align_w", (256, 512, 1, 1)), ("align_b", (256,)),
            ("up_w1", (1024, 256, 3, 3)), ("up_b1", (1024,)),
            ("up_w2", (128, 256, 1, 1)), ("up_b2", (128,)),
            ("re_w1", (32, 128, 3, 3)), ("re_b1", (32,)),
            ("re_w2", (128, 32, 3, 3)), ("re_b2", (128,)),
        ]}
        _, RE1_KEYS, RE2_Q = host_prep(zeros)
    return RE1_KEYS, RE2_Q


# ---------------------------------------------------------------- bass build
def pimg(ap):
    """View of a column-padded [128, 64*66] image (zero cols at x=0 and x=65)."""
    return ap.rearrange("p (y x) -> p y x", x=66)


def mm_views(src_ap, psum_ap, sy, sx, n):
    """(psum_out, rhs) for 8-row chunk n of a shifted conv tap on a col-padded
    source. Row range restricted by sy; columns handled by the zero pad.
    psum out is a contiguous 2D region."""
    y0 = max(0, -sy)
    y1 = 64 + min(0, -sy)
    r0 = max(8 * n, y0)
    r1 = min(8 * n + 8, y1)
    if r1 <= r0:
        return None, None
    v = pimg(src_ap)
    rhs = v[:, r0 + sy: r1 + sy, 1 + sx: 65 + sx]
    out = psum_ap[:, (r0 - 8 * n) * 64: (r1 - 8 * n) * 64]
    return out, rhs


def build():
    (re1_keys, re1_p3), re2_pairs = _mapping()
    nc = bacc.Bacc(trn_type="TRN2", target_bir_lowering=False, debug=False)

    BF16 = mybir.dt.bfloat16
    x_d = [nc.dram_tensor(n, [256, 4096], BF16, kind="ExternalInput")
           for n in ("x1", "x2")]
    wd = {}
    for name, shape, dt in [
        ("w1T0", [128, 128], F32), ("w1T1", [128, 128], F32),
        ("b1_0", [64, 1], F32), ("b1_1", [64, 1], F32),
        ("b1s0", [64, 1], F32), ("b1s1", [64, 1], F32),
        ("w2r0", [65, 2304], mybir.dt.bfloat16), ("w2r1", [65, 2304], mybir.dt.bfloat16),
        ("alignw", [128, 1024], mybir.dt.bfloat16), ("alignb", [128, 2], F32),
        ("fupw8h", [128, 9216], mybir.dt.float8e4), ("fupb", [128, 4], F32),
        ("fupw8l", [128, 9216], mybir.dt.float8e4),
        ("re1w8", [128, 2048], mybir.dt.float8e4), ("re1b", [128, 1], F32),
        ("re2w8", [128, 2048], mybir.dt.float8e4), ("re2b", [128, 1], F32),
        ("ident", [128, 128], mybir.dt.float8e4),
    ]:
        wd[name] = nc.dram_tensor(name, shape, dt, kind="ExternalInput")
    out_d = nc.dram_tensor("out", [128, 16384], mybir.dt.bfloat16,
                           kind="ExternalOutput")

    with tile.TileContext(nc) as tc, ExitStack() as ctx:
        wpool = ctx.enter_context(tc.tile_pool(name="w", bufs=1))
        big = ctx.enter_context(tc.tile_pool(name="big", bufs=5))
        flatp = ctx.enter_context(tc.tile_pool(name="flat", bufs=4))
        bnd = ctx.enter_context(tc.tile_pool(name="bnd", bufs=4))
        dgp = ctx.enter_context(tc.tile_pool(name="dg", bufs=2))
        f8p = ctx.enter_context(tc.tile_pool(name="f8", bufs=1))
        tiny = ctx.enter_context(tc.tile_pool(name="tiny", bufs=4))
        ps = ctx.enter_context(tc.tile_pool(name="ps", bufs=8, space="PSUM"))

        # persistent weights, spread across the three HWDGE queues (SP /
        # Activation / DVE) in rough order of first use so nothing blocks
        # the x loads or the first conditioning pass.
        wt = {}
        wq = {
            "w1T0": nc.sync, "b1_0": nc.sync, "b1s0": nc.sync, "w2r0": nc.sync,
            "ident": nc.scalar, "alignb": nc.gpsimd, "fupb": nc.gpsimd,
            "re1b": nc.gpsimd, "re2b": nc.gpsimd, "alignw": nc.gpsimd,
            "w1T1": nc.sync, "b1_1": nc.sync, "b1s1": nc.sync, "w2r1": nc.sync,
            "fupw8h": nc.gpsimd, "fupw8l": nc.gpsimd,
            "re1w8": nc.gpsimd, "re2w8": nc.gpsimd,
        }
        for name in wq:
            if name == "w2r1":
                continue  # allocated lazily into w2r0's slot (same tag)
            tag = "w2r0" if name == "w2r0" else name
            t = wpool.tile(list(wd[name].shape), wd[name].dtype, tag=tag)
            wt[name] = t

        def padtile(tag="big"):
            """Fresh [128, 64*66] F32R tile with zeroed pad columns."""
            t = big.tile([128, 4224], F32R, tag=tag)
            v = pimg(t[:].bitcast(F32))
            nc.vector.memset(v[:, :, 0:1], 0.0)
            nc.vector.memset(v[:, :, 65:66], 0.0)
            return t

        # x in: contiguous quarter DMAs into small flat tiles (16KB-contig
        # descriptors dodge the <512B strided-DMA 2x penalty), then ScalarE
        # repacks each quarter into the padded layout — with accum_out
        # producing the per-channel sums as a side effect.
        xq = [[nc.sync, nc.gpsimd], [nc.sync, nc.gpsimd]]
        xin = [[padtile(), padtile()], [padtile(), padtile()]]
        xgs = [[None, None], [None, None]]

        def load_x(i, c, q):
            xf = flatp.tile([128, 1024], mybir.dt.bfloat16, tag="flat")
            xq[i][c].dma_start(xf[:],
                               x_d[i].ap()[c * 128: (c + 1) * 128,
                                           q * 1024: (q + 1) * 1024])
            dst = pimg(xin[i][c][:])[:, q * 16: (q + 1) * 16, 1:65]
            src = xf[:].rearrange("p (y x) -> p y x", x=64)
            if xgs[i][c] is None:
                gq = tiny.tile([128, 4], F32, tag="gs4")
                xgs[i][c] = gq
            if c == 0:
                # ScalarE repack; accum_out yields the channel sums for free
                nc.scalar.activation(dst, src, AF.Copy,
                                     accum_out=xgs[i][c][:, q: q + 1])
            elif i == 0:
                # x1c1 on DVE, in parallel with ScalarE's x1c0 chain; the
                # cheap flat-quarter reduce gates gms, the repack follows
                nc.vector.tensor_reduce(xgs[i][c][:, q: q + 1], xf[:],
                                        axis=mybir.AxisListType.X, op=AL.add)
                nc.vector.tensor_scalar_add(dst, src, 0.0)
            else:
                # x2c1: cheap flat reduce on DVE; repack on Pool, which is
                # idle during dw1(0,*) while DVE carries depthwise taps
                nc.vector.tensor_reduce(xgs[i][c][:, q: q + 1], xf[:],
                                        axis=mybir.AxisListType.X, op=AL.add)
                nc.gpsimd.tensor_scalar_add(dst, src, 0.0)

        gms1 = [[None, None], [None, None]]

        def gms_of(i, c):
            g = tiny.tile([128, 1], F32, tag="gms")
            nc.vector.tensor_reduce(g[:], xgs[i][c][:],
                                    axis=mybir.AxisListType.X, op=AL.add)
            gms1[i][c] = g

        # All x quarters go through the SP queue: its SEQ serializes them in
        # this order, which also orders them on the (serial) DMA engines —
        # x1 lands first, x2 right behind, nothing jumps the line.
        # high_priority pins the x1 chain ahead of weight-DMA configs in
        # every scheduler tie so the extra flat buffers can't be hijacked.
        with tc.high_priority():
            for q in range(4):
                load_x(0, 0, q)
                load_x(0, 1, q)
        # conditioning weights for dsc1 right behind x1
        for name in ("w1T0", "b1_0", "b1s0", "w2r0", "ident"):
            wq[name].dma_start(wt[name][:], wd[name].ap())
        gms_of(0, 0)
        gms_of(0, 1)

        # ---------------- conditioning ----------------
        def cond_pg(d, gms):
            """gms: two [128,1] sums -> hv65 [65,1]: gelu'd hidden (x0.5
            folded) rows 0-63 plus a constant 1.0 row for the b2 fold."""
            pgt = ps.tile([128, 512], F32, tag="ps")
            pg = pgt[0:64, 0:1]
            for c in range(2):
                nc.tensor.matmul(pg, wt[f"w1T{d}"][:, c * 64: (c + 1) * 64],
                                 gms[c][:], start=(c == 0), stop=(c == 1))
            # u is O(1e-2) so the cubic term is negligible:
            # gelu(u) ~= 0.5 u (1 + tanh(0.79788 u)); the 0.5 lives in w2r.
            # Tanh shares the Exp act table -> no LoadActFuncSet swaps.
            th = tiny.tile([64, 1], F32, tag="th")
            nc.scalar.activation(th[:], pg, AF.Tanh, scale=0.7978845608028654,
                                 bias=wt[f"b1s{d}"][:])
            thp = tiny.tile([64, 1], F32, tag="thp")
            nc.vector.tensor_scalar_add(thp[:], th[:], 1.0)
            hv = tiny.tile([65, 1], mybir.dt.bfloat16, tag="hv")
            nc.vector.memset(hv[64:65, :], 1.0)
            nc.vector.scalar_tensor_tensor(hv[0:64, :], pg, wt[f"b1_{d}"][:],
                                           thp[:], AL.add, AL.mult)
            return hv

        def cond_pw(d, hv):
            """-> per chunk (ex [128,9] unnormalized softmax weights,
            c4 [128,1] = ex4+ssum center tap, rec [128,1] = 1/ssum).
            Normalization by rec happens in the psum-drain copy's scale."""
            res = []
            for c in range(2):
                pw = ps.tile([128, 512], F32, tag="ps")
                for k in range(9):
                    nc.tensor.matmul(pw[:, k: k + 1],
                                     wt[f"w2r{d}"][:, (c * 9 + k) * 128: (c * 9 + k + 1) * 128],
                                     hv[:], start=True, stop=True)
                # logits are O(1e-3) here (0.02-scale weights, gelu'd tiny
                # hidden): exp() cannot overflow, so no max subtraction.
                ex = tiny.tile([128, 9], F32, tag="ex")
                ssum = tiny.tile([128, 1], F32, tag="ssum")
                with tc.high_priority():
                    nc.scalar.activation(ex[:], pw[:, 0:9], AF.Exp, accum_out=ssum[:])
                rec = tiny.tile([128, 1], F32, tag="rec")
                c4 = tiny.tile([128, 1], F32, tag="c4")
                with tc.high_priority():
                    nc.vector.reciprocal(rec[:], ssum[:])
                    nc.vector.tensor_tensor(c4[:], ex[:, 4:5], ssum[:], AL.add)
                res.append((ex, c4, rec))
            return res

        def build_diags(ex, c4, rec):
            """center diag (its own tile so PE's first tap only waits on it)
            + 8 side diags split ScalarE/DVE; entries unnormalized (rec is
            applied at the psum drain)."""
            dgc = dgp.tile([128, 128], F32R, tag="dgc")
            dg = dgp.tile([128, 1024], F32R, tag="dg")
            with tc.high_priority():
                nc.scalar.activation(dgc[:], wt["ident"][:], AF.Copy, scale=c4[:])
                for j, k in enumerate((0, 1, 2, 3, 5, 6, 7, 8)):
                    dst = dg[:, j * 128: (j + 1) * 128]
                    if j % 2 == 0:
                        nc.scalar.activation(dst, wt["ident"][:], AF.Copy,
                                             scale=ex[:, k: k + 1])
                    else:
                        nc.vector.tensor_scalar(dst, wt["ident"][:],
                                                ex[:, k: k + 1], None, AL.mult)
            return dgc, dg

        SIDE = [0, 1, 2, 3, 5, 6, 7, 8]

        def depthwise_pe(src, dgs, dst_write, skip=(), hooks=None):
            """9-tap depthwise conv + residual on PE via diagonal matmuls.
            dst_write(n, psum): copy band n of the accumulated result out.
            Taps in `skip` are left for another engine to add afterwards.
            hooks[n]() is emitted before band n: the next chunk's cond /
            diag-build chains go here so their engine hops overlap this
            chunk's PE work instead of stalling PE at the boundary."""
            dgc, dg = dgs
            side = [k for k in SIDE if k not in skip]
            for n in range(8):
                if hooks and n in hooks:
                    hooks[n]()
                p = ps.tile([128, 512], F32, tag="ps")
                nc.tensor.matmul(p[:], dgc[:],
                                 pimg(src[:])[:, n * 8: (n + 1) * 8, 1:65],
                                 start=True, stop=False, skip_group_check=True)
                for ji, k in enumerate(side):
                    sy, sx = k // 3 - 1, k % 3 - 1
                    o, rhs = mm_views(src[:], p[:], sy, sx, n)
                    nc.tensor.matmul(o, dg[:, SIDE.index(k) * 128: (SIDE.index(k) + 1) * 128],
                                     rhs, start=False, stop=(ji == len(side) - 1),
                                     skip_group_check=True)
                dst_write(n, p)

        # ---------------- dsc stage ----------------
        # Emission order matters: PE executes its queue in order, so the tiny
        # conditioning matmuls for the NEXT (input, layer) are emitted between
        # depthwise chunks — their DVE/ScalarE chains then overlap the current
        # depthwise instead of stalling PE.
        mid = [[None, None], [None, None]]
        gms2 = [[None, None], [None, None]]
        y = [[None, None], [None, None]]

        def dw1(i, c, cw, dgs, hooks=None):
            ex, _, rec = cw[c]
            m = padtile()
            gs = tiny.tile([128, 8], F32, tag="gs")
            src_v = pimg(xin[i][c][:])

            def wr(n, p):
                # tap 7 (sy=+1) added by DVE straight into the psum between
                # the PE group and the ScalarE drain (unnormalized, like
                # PE's); for x2 (i=1) DVE also takes tap 1 — its repack work
                # is done by then. GPSIMD legally cannot touch PSUM.
                # high_priority: beat the x2 repacks for the DVE.
                pv = p[:].rearrange("p (y x) -> p y x", x=64)
                r0, r1 = n * 8, min((n + 1) * 8, 63)
                if r1 > r0:
                    with tc.high_priority():
                        nc.vector.scalar_tensor_tensor(
                            pv[:, 0: r1 - r0, :],
                            src_v[:, r0 + 1: r1 + 1, 1:65].bitcast(F32),
                            ex[:, 7: 8], pv[:, 0: r1 - r0, :], AL.mult, AL.add)
                if i == 1:
                    q0, q1 = max(n * 8, 1), (n + 1) * 8
                    with tc.high_priority():
                        nc.vector.scalar_tensor_tensor(
                            pv[:, q0 - n * 8: q1 - n * 8, :],
                            src_v[:, q0 - 1: q1 - 1, 1:65].bitcast(F32),
                            ex[:, 1: 2], pv[:, q0 - n * 8: q1 - n * 8, :],
                            AL.mult, AL.add)
                nc.scalar.activation(pimg(m[:])[:, n * 8: (n + 1) * 8, 1: 65],
                                     p[:].rearrange("p (y x) -> p y x", y=8),
                                     AF.Relu, scale=rec[:], accum_out=gs[:, n: n + 1])

            depthwise_pe(xin[i][c], dgs, wr, skip=(1, 7) if i == 1 else (7,),
                         hooks=hooks)
            g2 = tiny.tile([128, 1], F32, tag="gms")
            nc.vector.tensor_reduce(g2[:], gs[:], axis=mybir.AxisListType.X, op=AL.add)
            mid[i][c] = m
            gms2[i][c] = g2

        def dw2(i, c, cw, dgs, hooks=None):
            ex, _, rec = cw[c]
            yt = big.tile([128, 4096], mybir.dt.bfloat16, tag="big")

            def wr(n, p):
                nc.scalar.activation(yt[:, n * 512: (n + 1) * 512], p[:], AF.Copy,
                                     scale=rec[:])

            # taps 1/7 (sy=+-1) on DVE and tap 3 (sx=-1) on Pool, right
            # behind each ScalarE psum drain
            wns = {}
            for k in (1, 3, 7):
                wn = tiny.tile([128, 1], F32, tag="wn")
                nc.vector.tensor_scalar_mul(wn[:], ex[:, k: k + 1], rec[:])
                wns[k] = wn
            yv = yt[:].rearrange("p (y x) -> p y x", x=64)
            mv = pimg(mid[i][c][:])

            def wr2(n, p):
                wr(n, p)
                for k in (1, 7):
                    sy = k // 3 - 1
                    r0 = max(n * 8, -sy)
                    r1 = min((n + 1) * 8, 64 - max(0, sy))
                    nc.vector.scalar_tensor_tensor(
                        yv[:, r0: r1, :],
                        mv[:, r0 + sy: r1 + sy, 1:65].bitcast(F32),
                        wns[k][:], yv[:, r0: r1, :], AL.mult, AL.add)
                nc.gpsimd.scalar_tensor_tensor(
                    yv[:, n * 8: (n + 1) * 8, :],
                    mv[:, n * 8: (n + 1) * 8, 0:64].bitcast(F32),
                    wns[3][:], yv[:, n * 8: (n + 1) * 8, :], AL.mult, AL.add)

            depthwise_pe(mid[i][c], dgs, wr2, skip=(1, 3, 7), hooks=hooks)
            y[i][c] = yt

        hv = cond_pg(0, gms1[0])
        # x2 loads + sums, queued behind x1's
        for q in range(4):
            load_x(1, 0, q)
            load_x(1, 1, q)
        # Remaining weights held back (scheduler-time wait) so their
        # transfers cannot claim the serial DMA engines before the x loads.
        with tc.tile_wait_until(0.03):
            for name in ("w1T1", "b1_1", "b1s1", "alignb", "fupb",
                         "re1b", "re2b", "alignw", "re1w8", "re2w8",
                         "fupw8h", "fupw8l"):
                nc.scalar.dma_start(wt[name][:], wd[name].ap())
        cw = cond_pw(0, hv)
        # Each chunk's cond/diag chain is emitted via hooks inside the
        # PREVIOUS chunk, so its ScalarE/DVE hops overlap depthwise PE work
        # instead of stalling PE at chunk boundaries.
        st = {}
        d00 = build_diags(*cw[0])
        dw1(0, 0, cw, d00, hooks={
            2: lambda: (gms_of(1, 0), gms_of(1, 1)),
            4: lambda: st.update(hvb=cond_pg(0, gms1[1])),
            6: lambda: st.update(d01=build_diags(*cw[1])),
        })
        dw1(0, 1, cw, st["d01"], hooks={
            2: lambda: st.update(cwb=cond_pw(0, st["hvb"])),
            5: lambda: st.update(d10=build_diags(*st["cwb"][0])),
        })
        # dsc2's w2r reuses dsc1's buffer now that its readers are emitted
        w2r1t = wpool.tile([65, 2304], mybir.dt.bfloat16, tag="w2r0")
        wt["w2r1"] = w2r1t
        nc.scalar.dma_start(w2r1t[:], wd["w2r1"].ap())
        dw1(1, 0, st["cwb"], st["d10"], hooks={
            2: lambda: st.update(hv2=cond_pg(1, gms2[0])),
            4: lambda: st.update(cw2=cond_pw(1, st["hv2"])),
            6: lambda: st.update(d11=build_diags(*st["cwb"][1])),
        })
        dw1(1, 1, st["cwb"], st["d11"], hooks={
            3: lambda: st.update(d200=build_diags(*st["cw2"][0])),
        })
        dw2(0, 0, st["cw2"], st["d200"], hooks={
            2: lambda: st.update(hvb2=cond_pg(1, gms2[1])),
            4: lambda: st.update(cwb2=cond_pw(1, st["hvb2"])),
            6: lambda: st.update(d201=build_diags(*st["cw2"][1])),
        })
        dw2(0, 1, st["cw2"], st["d201"], hooks={
            3: lambda: st.update(d210=build_diags(*st["cwb2"][0])),
        })
        dw2(1, 0, st["cwb2"], st["d210"], hooks={
            3: lambda: st.update(d211=build_diags(*st["cwb2"][1])),
        })
        dw2(1, 1, st["cwb2"], st["d211"])

        # ---------------- align 1x1 (2C -> C) -> fp8 hi/lo fus versions ----------------
        # fus is stored as three e4m3 images, both kc chunks in ONE tile
        # (kc-major) so a DoubleRow rhs AP can address the pair:
        #   fus8h = e4m3(A), fus8s = e4m3(A)/16, fus8l = A - e4m3(A)
        FP8 = mybir.dt.float8e4

        def fp8img(tag):
            t = f8p.tile([128, 8448], FP8, tag=tag)
            v = t[:].rearrange("p (kc y x) -> p kc y x", kc=2, x=66)
            nc.vector.memset(v[:, :, :, 0:1], 0.0)
            nc.vector.memset(v[:, :, :, 65:66], 0.0)
            return t

        def fp8img1(tag):
            t = f8p.tile([128, 4224], FP8, tag=tag)
            v = t[:].rearrange("p (y x) -> p y x", x=66)
            nc.vector.memset(v[:, :, 0:1], 0.0)
            nc.vector.memset(v[:, :, 65:66], 0.0)
            return t

        fus8h = fp8img("f8h")
        fus8l = fp8img("f8l")
        for mc in range(2):
            for n in range(8):
                p = ps.tile([128, 512], F32, tag="ps")
                for kc in range(4):
                    rhs = y[kc // 2][kc % 2][:, n * 512: (n + 1) * 512]
                    nc.tensor.matmul(
                        p[:], wt["alignw"][:, (kc * 2 + mc) * 128: (kc * 2 + mc + 1) * 128],
                        rhs, start=(kc == 0), stop=(kc == 3))
                pv = p[:].rearrange("p (y x) -> p y x", y=8)
                roff = mc * 4224 + n * 8 * 66
                hv_ = fus8h[:, roff: roff + 528].rearrange("p (y x) -> p y x", x=66)[:, :, 1:65]
                lv_ = fus8l[:, roff: roff + 528].rearrange("p (y x) -> p y x", x=66)[:, :, 1:65]
                nc.scalar.activation(hv_, pv, AF.Identity, bias=wt["alignb"][:, mc: mc + 1])
                nc.vector.scalar_tensor_tensor(lv_, pv, wt["alignb"][:, mc: mc + 1],
                                               hv_, AL.add, AL.subtract)

        # ---------------- fused upsampler: per-phase 3x3 conv 256->128 ----------------
        # fp8 e4m3 DoubleRow (both kc in one matmul, 0.5 cyc/row), 3 passes:
        #   Wh@Ah + (Wl*16)@(Ah/16) + Wh@(A-Ah); weights carry x64, /64 at drain
        taps = [(1, 1)] + [(dy, dx) for dy in range(3) for dx in range(3) if (dy, dx) != (1, 1)]

        def dr_views(src_ap, psum_ap, sy, sx, n):
            y0 = max(0, -sy)
            y1 = 64 + min(0, -sy)
            r0 = max(8 * n, y0)
            r1 = min(8 * n + 8, y1)
            v = src_ap.rearrange("p (kc y x) -> p kc y x", kc=2, x=66)
            rhs = v[:, :, r0 + sy: r1 + sy, 1 + sx: 65 + sx]
            out = psum_ap[:, (r0 - 8 * n) * 64: (r1 - 8 * n) * 64]
            return out, rhs

        def pair_ap(base3d, delta):
            u = base3d.unsqueeze(1).copy()
            u.ap[1] = [delta, 2]
            return u

        up2p = []
        up8 = []
        for p4 in range(4):
            up2p.append(padtile())
            up8.append(fp8img1(f"u8{p4}"))

        def fup_band(p4, n):
            p = ps.tile([128, 512], F32, tag="ps")
            first = True
            for (dy, dx) in taps:
                sy, sx = dy - 1, dx - 1
                toff = (p4 * 9 + dy * 3 + dx) * 256
                wh = wt["fupw8h"][:, toff: toff + 256].rearrange(
                    "p (k m) -> p k m", k=2)
                wl = wt["fupw8l"][:, toff: toff + 256].rearrange(
                    "p (k m) -> p k m", k=2)
                for wtile, src in ((wh, fus8h), (wl, fus8h), (wh, fus8l)):
                    o, rhs = dr_views(src[:], p[:], sy, sx, n)
                    nc.tensor.matmul(o, wtile, rhs, start=first,
                                     stop=((dy, dx) == taps[-1] and src is fus8l),
                                     perf_mode=mybir.MatmulPerfMode.DoubleRow,
                                     skip_group_check=True)
                    first = False
            pv = p[:].rearrange("p (y x) -> p y x", y=8)
            fb = pimg(up2p[p4][:])[:, n * 8: (n + 1) * 8, 1:65]
            nc.scalar.activation(fb, pv, AF.Identity, scale=0.015625,
                                 bias=wt["fupb"][:, p4: p4 + 1])
            # fp8 copy of up (re2b backed out) for re1's DoubleRow rhs;
            # runs on DVE, which is otherwise idle during the fup window
            fbf = pimg(up2p[p4][:].bitcast(F32))[:, n * 8: (n + 1) * 8, 1:65]
            u8b = pimg(up8[p4][:])[:, n * 8: (n + 1) * 8, 1:65]
            nc.vector.tensor_scalar(u8b, fbf, wt["re2b"][:], None,
                                    AL.subtract)

        # ---------------- re1 (polyphase 3x3, M-packed, fp8 DR 1-pass) ----------------
        s64 = tiny.tile([128, 1], F32, tag="s64")
        nc.vector.memset(s64[:], 0.015625)

        re1t = fp8img1("re18")

        def re1_n(n):
            p = ps.tile([128, 512], F32, tag="ps")
            for pi, (qy, pa, pb) in enumerate(re1_p3):
                pinA, qxA, _ = pa
                pinB, qxB, _ = pb
                assert pinB == pinA and qxB - qxA == 1
                o, rhs = mm_views(up8[pinA][:], p[:], qy, qxA, n)
                u = rhs.unsqueeze(1).copy()
                u.ap[1] = [1, 2]
                nc.tensor.matmul(o, wt["re1w8"][:, pi * 256: (pi + 1) * 256]
                                 .rearrange("p (k m) -> p k m", k=2),
                                 u, start=(pi == 0), stop=(pi == len(re1_p3) - 1),
                                 perf_mode=mybir.MatmulPerfMode.DoubleRow,
                                 skip_group_check=True)
            pv = p[:].rearrange("p (y x) -> p y x", y=8)
            fb = pimg(re1t[:])[:, n * 8: (n + 1) * 8, 1:65]
            nc.scalar.activation(fb, pv, AF.Relu, scale=0.015625,
                                 bias=wt["re1b"][:])

        # ---------------- re2 (polyphase 3x3, fp8 DR 1-pass) + residual + out ----------------
        def re2_n(n, split=False):
            pss = []
            for p4 in range(4):
                p = ps.tile([128, 512], F32, tag="ps")
                for pj, (qy, qxA, qxB) in enumerate(re2_pairs[p4]):
                    assert qxB - qxA == 1
                    o, rhs = mm_views(re1t[:], p[:], qy, qxA, n)
                    u = rhs.unsqueeze(1).copy()
                    u.ap[1] = [1, 2]
                    nc.tensor.matmul(
                        o, wt["re2w8"][:, (p4 * 2 + pj) * 256:
                                       (p4 * 2 + pj + 1) * 256]
                        .rearrange("p (k m) -> p k m", k=2),
                        u, start=(pj == 0), stop=(pj == 1),
                        perf_mode=mybir.MatmulPerfMode.DoubleRow,
                        skip_group_check=True)
                pss.append(p)
            band = bnd.tile([128, 2048], mybir.dt.bfloat16, tag="bnd")
            bv = band[:].rearrange("p (y r x s) -> p y r x s", y=8, r=2, s=2)
            if not split:
                for p4 in range(4):
                    r, s = p4 // 2, p4 % 2
                    up_v = pimg(up2p[p4][:].bitcast(F32))[:, n * 8: (n + 1) * 8, 1:65]
                    out_v = bv[:, :, r, :, s]
                    psv = pss[p4][:].rearrange("p (y x) -> p y x", y=8)
                    if p4 < 3:
                        # up2p carries +re2b: psum/64 + up2p is the full output
                        nc.vector.scalar_tensor_tensor(
                            out_v, psv, s64[:], up_v, AL.mult, AL.add)
                    else:
                        # ScalarE drains psum/64; Pool adds the residual (+re2b)
                        nc.scalar.activation(out_v, psv, AF.Identity,
                                             scale=0.015625)
                        nc.gpsimd.tensor_tensor(out_v, out_v, up_v, AL.add)
                nc.sync.dma_start(
                    out_d.ap()[:, n * 2048: (n + 1) * 2048], band[:])
                return
            # tail bands: half-band drains spread across DVE/ACT/Pool and two
            # half DMAs, so the final store only waits on the last half
            for h in range(2):
                for p4 in range(4):
                    r, s = p4 // 2, p4 % 2
                    up_v = pimg(up2p[p4][:].bitcast(F32))[
                        :, n * 8 + h * 4: n * 8 + (h + 1) * 4, 1:65]
                    out_v = bv[:, h * 4: (h + 1) * 4, r, :, s]
                    psv = pss[p4][:].rearrange("p (y x) -> p y x", y=8)[
                        :, h * 4: (h + 1) * 4, :]
                    if p4 < 2:
                        nc.vector.scalar_tensor_tensor(
                            out_v, psv, s64[:], up_v, AL.mult, AL.add)
                    else:
                        nc.scalar.activation(out_v, psv, AF.Identity,
                                             scale=0.015625)
                        if p4 == 2:
                            nc.gpsimd.tensor_tensor(out_v, out_v, up_v, AL.add)
                        else:
                            nc.vector.tensor_tensor(out_v, out_v, up_v, AL.add)
                nc.sync.dma_start(
                    out_d.ap()[:, n * 2048 + h * 1024: n * 2048 + (h + 1) * 1024],
                    band[:, h * 1024: (h + 1) * 1024])

        # band-major driver: all four fup phases of band n, then re1(n-1)
        # and re2(n-2) — each re stage trails by one band because it reads
        # its input through row 8n+8. Spreading re1/re2 into the fup window
        # lets their drain chains ride on otherwise-idle DVE/ACT/Pool time
        # instead of forming a drain-bound tail phase.
        for n in range(8):
            for p4 in range(4):
                fup_band(p4, n)
            if n >= 1:
                re1_n(n - 1)
            if n >= 2:
                re2_n(n - 2)
        re1_n(7)
        re2_n(6, split=True)
        re2_n(7, split=True)

    nc.compile()
    return nc


_NC = None


def _get_nc():
    global _NC
    if _NC is None:
        _NC = build()
    return _NC


def make_in_maps(inputs):
    import ml_dtypes
    w, _, _ = host_prep(inputs)
    x1 = np.ascontiguousarray(np.asarray(inputs["x1"], np.float32).reshape(NC, 256, 4096)
                              .astype(ml_dtypes.bfloat16))
    x2 = np.ascontiguousarray(np.asarray(inputs["x2"], np.float32).reshape(NC, 256, 4096)
                              .astype(ml_dtypes.bfloat16))
    in_maps = []
    for i in range(NC):
        m = {"x1": x1[i], "x2": x2[i]}
        m.update(w)
        in_maps.append(m)
    return in_maps


def kernel(**inputs):
    nc = _get_nc()
    in_maps = make_in_maps(inputs)
    res = run_bass_kernel_spmd(nc, in_maps, core_ids=list(range(NC)))
    out = np.stack([res.results[i]["out"].reshape(128, 128, 128) for i in range(NC)])
    return out.astype(np.float32)

